# revision 24
# baseline (speedup 1.0000x reference)
"""Trainium2 Bass kernel for nn_AqtDotGeneral_19481971655318.

Computes the AQT-style int8 quantized matmul:
    lhs_scale = absmax(lhs, axis=K) / 127        # [B,S,1] per row
    rhs_scale = absmax(rhs, axis=K) / 127        # [1,N]   per column
    q_lhs = round(lhs / lhs_scale)  (int8 range)
    q_rhs = round(rhs / rhs_scale)
    out = (q_lhs @ q_rhs) * lhs_scale * rhs_scale

Sharding: data-parallel over B*S rows across 8 cores (4096 rows each);
rhs replicated. Per-core dataflow:
  - lhs row-tile pairs [128,2,1024] f32 DMA in (1MB transfers);
    fused DVE absmax-reduce per pair
  - quantize: ACT (x*rs + M) with per-partition scale, DVE (y - M)
    -> integer-valued bf16 (M = 1.5*2^23 magic number gives
    round-half-even, matching jnp.round)
  - xbar DMA transpose -> lhsT [K,M] bf16 tiles
  - TensorE bf16 matmuls accumulate in PSUM over K (exact: integer
    products up to 127*127 are exact in bf16*bf16->f32-accum)
  - per-column rhs scale is folded into q_rhs (bf16), so dequant is one
    ScalarE pass: out = psum * row_scale, PSUM->SBUF, then DMA out.
"""

import numpy as np

N_CORES = 8
B, S, K, N = 4, 8192, 1024, 1024
ROWS_TOTAL = B * S
ROWS_PER_CORE = ROWS_TOTAL // N_CORES  # 4096
P = 128
ROW_TILES = ROWS_PER_CORE // P  # 32
KT = K // P  # 8 k-tiles
NB = N // P  # 8 n-blocks
MAGIC = 12582912.0  # 1.5 * 2**23: float32 round-to-nearest-even trick
INV127 = 1.0 / 127.0

_CACHE = {}


def _build():
    import concourse.mybir as mybir
    import concourse.tile as tile
    from concourse import bacc
    from concourse.masks import make_identity

    f32 = mybir.dt.float32
    bf16 = mybir.dt.bfloat16

    nc = bacc.Bacc(None, target_bir_lowering=False, debug=False)
    lhs_d = nc.declare_dram_parameter("lhs", [ROWS_PER_CORE, K], f32, isOutput=False)
    # rhs is shipped PRE-TRANSPOSED from the host: [N, K] f32
    rhs_d = nc.declare_dram_parameter("rhs", [N, K], f32, isOutput=False)
    out_d = nc.declare_dram_parameter("out", [ROWS_PER_CORE, N], f32, isOutput=True)

    with tile.TileContext(nc) as tc:
        with (
            tc.tile_pool(name="qrhs", bufs=1) as qrhs_pool,
            tc.tile_pool(name="lhsx", bufs=6) as lhsx_pool,
            tc.tile_pool(name="scales", bufs=4) as scales_pool,
            tc.tile_pool(name="ybuf", bufs=6) as y_pool,
            tc.tile_pool(name="qbuf", bufs=6) as q_pool,
            tc.tile_pool(name="qlt", bufs=10) as qlt_pool,
            tc.tile_pool(name="obuf", bufs=2) as o_pool,
            tc.tile_pool(name="mm_psum", bufs=2, space="PSUM") as mm_psum,
            tc.tile_pool(name="tp_psum", bufs=2, space="PSUM") as tp_psum,
        ):
            # ------- rhs prep (replicated: every core quantizes the full
            # rhs; runs in the startup window while PE/DMA are idle) -----
            q_rhs = qrhs_pool.tile([P, KT, N], bf16, name="q_rhs")
            ident = qrhs_pool.tile([P, P], f32, name="ident")
            make_identity(nc, ident)
            identb = qrhs_pool.tile([P, P], bf16, name="identb")
            nc.vector.tensor_copy(out=identb, in_=ident)
            with (
                tc.tile_pool(name="rhsbuf", bufs=2) as rhsbuf_pool,
                tc.tile_pool(name="rhs_misc", bufs=1) as rhs_misc,
            ):
                s_col = rhs_misc.tile([P, NB], f32, name="s_col")
                rs_col = rhs_misc.tile([P, NB], f32, name="rs_col")
                namax = rhs_misc.tile([P, NB], f32, name="namax")
                rhsTs = []
                for nb in range(NB):
                    rhsT_loc = rhsbuf_pool.tile(
                        [P, K], f32, name="rhsT_loc", tag="rhsT_loc", bufs=NB)
                    nc.sync.dma_start(
                        out=rhsT_loc, in_=rhs_d[nb * P:(nb + 1) * P, :])
                    nc.vector.tensor_reduce(
                        out=namax[:, nb:nb + 1], in_=rhsT_loc,
                        axis=mybir.AxisListType.X, op=mybir.AluOpType.max,
                        apply_absolute_value=True,
                    )
                    rhsTs.append(rhsT_loc)
                nc.vector.tensor_scalar_mul(s_col, namax, INV127)
                nc.vector.reciprocal(rs_col, s_col)

                for nb in range(NB):
                    # quantize + fold col-scale in transposed layout
                    yT = y_pool.tile([P, K], f32, name="y", tag="y")
                    nc.scalar.activation(
                        out=yT, in_=rhsTs[nb],
                        func=mybir.ActivationFunctionType.Copy,
                        bias=MAGIC, scale=rs_col[:, nb:nb + 1],
                    )
                    qiT = y_pool.tile([P, K], f32, name="y", tag="y")
                    nc.vector.tensor_scalar(
                        out=qiT, in0=yT, scalar1=MAGIC, scalar2=None,
                        op0=mybir.AluOpType.subtract,
                    )
                    qsT = q_pool.tile([P, K], bf16, name="q", tag="q")
                    nc.scalar.activation(
                        out=qsT, in_=qiT,
                        func=mybir.ActivationFunctionType.Copy,
                        bias=0.0, scale=s_col[:, nb:nb + 1],
                    )
                    # transpose back to [K, n] via PE, land in q_rhs
                    psb = tp_psum.tile([P, KT * P], bf16, name="psb2", tag="psb2")
                    for b in range(KT):
                        nc.tensor.transpose(
                            psb[:, b * P:(b + 1) * P],
                            qsT[:, b * P:(b + 1) * P], identb)
                    nc.vector.tensor_copy(
                        out=q_rhs[:, :, nb * P:(nb + 1) * P],
                        in_=psb.rearrange("p (b m) -> p b m", b=KT))

            # ---------------- lhs main loop ------------------------------
            # row-tile pairs: one 1MB DMA + one fused reduce per pair,
            # then a uniform per-tile quant->transpose->matmul pipeline.
            # Trace order is software-pipelined: the load/reduce/scale
            # stage runs LA pairs ahead of the compute stage so the
            # scheduler prefetches.
            PAIRS = ROW_TILES // 2
            LA = 6
            staged = {}

            def stage_load(pi):
                xp = lhsx_pool.tile([P, 2, K], f32, name="xp", tag="xp")
                nc.scalar.dma_start(
                    out=xp,
                    in_=lhs_d[pi * 2 * P:(pi + 1) * 2 * P, :].rearrange(
                        "(t p) k -> p t k", p=P),
                )
                amax = scales_pool.tile([P, 2], f32, name="amax", tag="amax")
                nc.vector.tensor_reduce(
                    out=amax, in_=xp,
                    axis=mybir.AxisListType.X, op=mybir.AluOpType.max,
                    apply_absolute_value=True,
                )
                s_row = scales_pool.tile([P, 2], f32, name="s_row", tag="s_row")
                nc.vector.tensor_scalar_mul(s_row, amax, INV127)
                rs_row = scales_pool.tile(
                    [P, 2], f32, name="rs_row", tag="rs_row")
                nc.vector.reciprocal(rs_row, s_row)
                staged[pi] = (xp, s_row, rs_row)

            def stage_compute(pi):
                xp, s_row, rs_row = staged.pop(pi)
                o_pair = o_pool.tile([P, 2, N], f32, name="opair", tag="opair")
                for h in range(2):
                    y = y_pool.tile([P, K], f32, name="y", tag="y")
                    nc.scalar.activation(
                        out=y, in_=xp[:, h, :],
                        func=mybir.ActivationFunctionType.Copy,
                        bias=MAGIC, scale=rs_row[:, h:h + 1],
                    )
                    q = q_pool.tile([P, K], bf16, name="q", tag="q")
                    nc.vector.tensor_scalar(
                        out=q, in0=y, scalar1=MAGIC, scalar2=None,
                        op0=mybir.AluOpType.subtract,
                    )
                    qlT = qlt_pool.tile([P, KT, P], bf16, name="qlT", tag="qlT")
                    psb = tp_psum.tile([P, KT * P], bf16, name="psb2", tag="psb2")
                    for b in range(KT):
                        nc.tensor.transpose(
                            psb[:, b * P:(b + 1) * P], q[:, b * P:(b + 1) * P],
                            identb)
                    nc.vector.tensor_copy(
                        out=qlT.rearrange("p b m -> p (b m)"), in_=psb)

                    ps0 = mm_psum.tile([P, 512], f32, name="ps0", tag="ps0")
                    ps1 = mm_psum.tile([P, 512], f32, name="ps1", tag="ps1")
                    for b in range(KT):
                        nc.tensor.matmul(
                            ps0, qlT[:, b, :], q_rhs[:, b, 0:512],
                            start=(b == 0), stop=(b == KT - 1),
                        )
                        nc.tensor.matmul(
                            ps1, qlT[:, b, :], q_rhs[:, b, 512:1024],
                            start=(b == 0), stop=(b == KT - 1),
                        )
                    nc.scalar.activation(
                        out=o_pair[:, h, 0:512], in_=ps0,
                        func=mybir.ActivationFunctionType.Copy,
                        bias=0.0, scale=s_row[:, h:h + 1],
                    )
                    nc.scalar.activation(
                        out=o_pair[:, h, 512:1024], in_=ps1,
                        func=mybir.ActivationFunctionType.Copy,
                        bias=0.0, scale=s_row[:, h:h + 1],
                    )
                nc.sync.dma_start(
                    out=out_d[pi * 2 * P:(pi + 1) * 2 * P, :].rearrange(
                        "(t p) k -> p t k", p=P),
                    in_=o_pair,
                )

            for pi in range(PAIRS + LA):
                if pi < PAIRS:
                    stage_load(pi)
                if pi >= LA:
                    stage_compute(pi - LA)

    nc.compile()
    return nc


def _get_nc():
    if "nc" not in _CACHE:
        _CACHE["nc"] = _build()
    return _CACHE["nc"]


def kernel(lhs: np.ndarray, rhs: np.ndarray) -> np.ndarray:
    from concourse.bass_utils import run_bass_kernel_spmd

    nc = _get_nc()
    lhs_flat = np.ascontiguousarray(lhs.reshape(ROWS_TOTAL, K), dtype=np.float32)
    rhsT = np.ascontiguousarray(rhs.T, dtype=np.float32)
    in_maps = [
        {
            "lhs": lhs_flat[c * ROWS_PER_CORE:(c + 1) * ROWS_PER_CORE],
            "rhs": rhsT,
        }
        for c in range(N_CORES)
    ]
    res = run_bass_kernel_spmd(nc, in_maps, core_ids=list(range(N_CORES)))
    out = np.concatenate([res.results[c]["out"] for c in range(N_CORES)], axis=0)
    return out.reshape(B, S, N)


# revision 25
# speedup vs baseline: 1.0042x; 1.0042x over previous
"""Trainium2 Bass kernel for nn_AqtDotGeneral_19481971655318.

Computes the AQT-style int8 quantized matmul:
    lhs_scale = absmax(lhs, axis=K) / 127        # [B,S,1] per row
    rhs_scale = absmax(rhs, axis=K) / 127        # [1,N]   per column
    q_lhs = round(lhs / lhs_scale)  (int8 range)
    q_rhs = round(rhs / rhs_scale)
    out = (q_lhs @ q_rhs) * lhs_scale * rhs_scale

Sharding: data-parallel over B*S rows across 8 cores (4096 rows each);
rhs replicated. Per-core dataflow:
  - lhs row-tile pairs [128,2,1024] f32 DMA in (1MB transfers);
    fused DVE absmax-reduce per pair
  - quantize: ACT (x*rs + M) with per-partition scale, DVE (y - M)
    -> integer-valued bf16 (M = 1.5*2^23 magic number gives
    round-half-even, matching jnp.round)
  - xbar DMA transpose -> lhsT [K,M] bf16 tiles
  - TensorE bf16 matmuls accumulate in PSUM over K (exact: integer
    products up to 127*127 are exact in bf16*bf16->f32-accum)
  - per-column rhs scale is folded into q_rhs (bf16), so dequant is one
    ScalarE pass: out = psum * row_scale, PSUM->SBUF, then DMA out.
"""

import numpy as np

N_CORES = 8
B, S, K, N = 4, 8192, 1024, 1024
ROWS_TOTAL = B * S
ROWS_PER_CORE = ROWS_TOTAL // N_CORES  # 4096
P = 128
ROW_TILES = ROWS_PER_CORE // P  # 32
KT = K // P  # 8 k-tiles
NB = N // P  # 8 n-blocks
MAGIC = 12582912.0  # 1.5 * 2**23: float32 round-to-nearest-even trick
INV127 = 1.0 / 127.0

_CACHE = {}


def _build():
    import concourse.mybir as mybir
    import concourse.tile as tile
    from concourse import bacc
    from concourse.masks import make_identity

    f32 = mybir.dt.float32
    bf16 = mybir.dt.bfloat16

    nc = bacc.Bacc(None, target_bir_lowering=False, debug=False)
    lhs_d = nc.declare_dram_parameter("lhs", [ROWS_PER_CORE, K], f32, isOutput=False)
    # rhs is shipped PRE-TRANSPOSED from the host: [N, K] f32
    rhs_d = nc.declare_dram_parameter("rhs", [N, K], f32, isOutput=False)
    out_d = nc.declare_dram_parameter("out", [ROWS_PER_CORE, N], f32, isOutput=True)

    with tile.TileContext(nc) as tc:
        with (
            tc.tile_pool(name="qrhs", bufs=1) as qrhs_pool,
            tc.tile_pool(name="lhsx", bufs=6) as lhsx_pool,
            tc.tile_pool(name="scales", bufs=4) as scales_pool,
            tc.tile_pool(name="ybuf", bufs=6) as y_pool,
            tc.tile_pool(name="qbuf", bufs=6) as q_pool,
            tc.tile_pool(name="qlt", bufs=10) as qlt_pool,
            tc.tile_pool(name="obuf", bufs=2) as o_pool,
            tc.tile_pool(name="mm_psum", bufs=2, space="PSUM") as mm_psum,
            tc.tile_pool(name="tp_psum", bufs=2, space="PSUM") as tp_psum,
        ):
            # ------- rhs prep (replicated: every core quantizes the full
            # rhs; runs in the startup window while PE/DMA are idle) -----
            q_rhs = qrhs_pool.tile([P, KT, N], bf16, name="q_rhs")
            ident = qrhs_pool.tile([P, P], f32, name="ident")
            make_identity(nc, ident)
            identb = qrhs_pool.tile([P, P], bf16, name="identb")
            nc.vector.tensor_copy(out=identb, in_=ident)
            with (
                tc.tile_pool(name="rhsbuf", bufs=2) as rhsbuf_pool,
                tc.tile_pool(name="rhs_misc", bufs=1) as rhs_misc,
            ):
                s_col = rhs_misc.tile([P, NB], f32, name="s_col")
                rs_col = rhs_misc.tile([P, NB], f32, name="rs_col")
                namax = rhs_misc.tile([P, NB], f32, name="namax")
                rhsTs = []
                for nb in range(NB):
                    rhsT_loc = rhsbuf_pool.tile(
                        [P, K], f32, name="rhsT_loc", tag="rhsT_loc", bufs=NB)
                    nc.sync.dma_start(
                        out=rhsT_loc, in_=rhs_d[nb * P:(nb + 1) * P, :])
                    nc.vector.tensor_reduce(
                        out=namax[:, nb:nb + 1], in_=rhsT_loc,
                        axis=mybir.AxisListType.X, op=mybir.AluOpType.max,
                        apply_absolute_value=True,
                    )
                    rhsTs.append(rhsT_loc)
                nc.vector.tensor_scalar_mul(s_col, namax, INV127)
                nc.vector.reciprocal(rs_col, s_col)

                for nb in range(NB):
                    # quantize + fold col-scale in transposed layout
                    yT = y_pool.tile([P, K], f32, name="y", tag="y")
                    nc.scalar.activation(
                        out=yT, in_=rhsTs[nb],
                        func=mybir.ActivationFunctionType.Copy,
                        bias=MAGIC, scale=rs_col[:, nb:nb + 1],
                    )
                    qiT = y_pool.tile([P, K], f32, name="y", tag="y")
                    nc.vector.tensor_scalar(
                        out=qiT, in0=yT, scalar1=MAGIC, scalar2=None,
                        op0=mybir.AluOpType.subtract,
                    )
                    qsT = q_pool.tile([P, K], bf16, name="q", tag="q")
                    nc.scalar.activation(
                        out=qsT, in_=qiT,
                        func=mybir.ActivationFunctionType.Copy,
                        bias=0.0, scale=s_col[:, nb:nb + 1],
                    )
                    # transpose back to [K, n] via PE, land in q_rhs
                    psb = tp_psum.tile([P, KT * P], bf16, name="psb2", tag="psb2")
                    for b in range(KT):
                        nc.tensor.transpose(
                            psb[:, b * P:(b + 1) * P],
                            qsT[:, b * P:(b + 1) * P], identb)
                    nc.vector.tensor_copy(
                        out=q_rhs[:, :, nb * P:(nb + 1) * P],
                        in_=psb.rearrange("p (b m) -> p b m", b=KT))

            # ---------------- lhs main loop ------------------------------
            # row-tile pairs: one 1MB DMA + one fused reduce per pair,
            # then a uniform per-tile quant->transpose->matmul pipeline.
            # Trace order is software-pipelined: the load/reduce/scale
            # stage runs LA pairs ahead of the compute stage so the
            # scheduler prefetches.
            PAIRS = ROW_TILES // 2
            LA = 6
            staged = {}

            def stage_load(pi):
                xp = lhsx_pool.tile([P, 2, K], f32, name="xp", tag="xp")
                nc.scalar.dma_start(
                    out=xp,
                    in_=lhs_d[pi * 2 * P:(pi + 1) * 2 * P, :].rearrange(
                        "(t p) k -> p t k", p=P),
                )
                amax = scales_pool.tile([P, 2], f32, name="amax", tag="amax")
                nc.vector.tensor_reduce(
                    out=amax, in_=xp,
                    axis=mybir.AxisListType.X, op=mybir.AluOpType.max,
                    apply_absolute_value=True,
                )
                s_row = scales_pool.tile([P, 2], f32, name="s_row", tag="s_row")
                nc.vector.tensor_scalar_mul(s_row, amax, INV127)
                rs_row = scales_pool.tile(
                    [P, 2], f32, name="rs_row", tag="rs_row")
                nc.vector.reciprocal(rs_row, s_row)
                staged[pi] = (xp, s_row, rs_row)

            def stage_compute(pi):
                xp, s_row, rs_row = staged.pop(pi)
                o_pair = o_pool.tile([P, 2, N], f32, name="opair", tag="opair")
                for h in range(2):
                    y = y_pool.tile([P, K], f32, name="y", tag="y")
                    nc.scalar.activation(
                        out=y, in_=xp[:, h, :],
                        func=mybir.ActivationFunctionType.Copy,
                        bias=MAGIC, scale=rs_row[:, h:h + 1],
                    )
                    q = q_pool.tile([P, K], bf16, name="q", tag="q")
                    nc.vector.tensor_scalar(
                        out=q, in0=y, scalar1=MAGIC, scalar2=None,
                        op0=mybir.AluOpType.subtract,
                    )
                    qlT = qlt_pool.tile([P, KT, P], bf16, name="qlT", tag="qlT")
                    psb = tp_psum.tile([P, KT * P], bf16, name="psb2", tag="psb2")
                    for b in range(KT):
                        nc.tensor.transpose(
                            psb[:, b * P:(b + 1) * P], q[:, b * P:(b + 1) * P],
                            identb)
                    nc.vector.tensor_copy(
                        out=qlT.rearrange("p b m -> p (b m)"), in_=psb)

                    ps0 = mm_psum.tile([P, 512], f32, name="ps0", tag="ps0")
                    ps1 = mm_psum.tile([P, 512], f32, name="ps1", tag="ps1")
                    for b in range(KT):
                        nc.tensor.matmul(
                            ps0, qlT[:, b, :], q_rhs[:, b, 0:512],
                            start=(b == 0), stop=(b == KT - 1),
                        )
                    for b in range(KT):
                        nc.tensor.matmul(
                            ps1, qlT[:, b, :], q_rhs[:, b, 512:1024],
                            start=(b == 0), stop=(b == KT - 1),
                        )
                    nc.scalar.activation(
                        out=o_pair[:, h, 0:512], in_=ps0,
                        func=mybir.ActivationFunctionType.Copy,
                        bias=0.0, scale=s_row[:, h:h + 1],
                    )
                    nc.scalar.activation(
                        out=o_pair[:, h, 512:1024], in_=ps1,
                        func=mybir.ActivationFunctionType.Copy,
                        bias=0.0, scale=s_row[:, h:h + 1],
                    )
                nc.sync.dma_start(
                    out=out_d[pi * 2 * P:(pi + 1) * 2 * P, :].rearrange(
                        "(t p) k -> p t k", p=P),
                    in_=o_pair,
                )

            for pi in range(PAIRS + LA):
                if pi < PAIRS:
                    stage_load(pi)
                if pi >= LA:
                    stage_compute(pi - LA)

    nc.compile()
    return nc


def _get_nc():
    if "nc" not in _CACHE:
        _CACHE["nc"] = _build()
    return _CACHE["nc"]


def kernel(lhs: np.ndarray, rhs: np.ndarray) -> np.ndarray:
    from concourse.bass_utils import run_bass_kernel_spmd

    nc = _get_nc()
    lhs_flat = np.ascontiguousarray(lhs.reshape(ROWS_TOTAL, K), dtype=np.float32)
    rhsT = np.ascontiguousarray(rhs.T, dtype=np.float32)
    in_maps = [
        {
            "lhs": lhs_flat[c * ROWS_PER_CORE:(c + 1) * ROWS_PER_CORE],
            "rhs": rhsT,
        }
        for c in range(N_CORES)
    ]
    res = run_bass_kernel_spmd(nc, in_maps, core_ids=list(range(N_CORES)))
    out = np.concatenate([res.results[c]["out"] for c in range(N_CORES)], axis=0)
    return out.reshape(B, S, N)


# revision 27
# speedup vs baseline: 1.1407x; 1.1359x over previous
"""Trainium2 Bass kernel for nn_AqtDotGeneral_19481971655318.

Computes the AQT-style int8 quantized matmul:
    lhs_scale = absmax(lhs, axis=K) / 127        # [B,S,1] per row
    rhs_scale = absmax(rhs, axis=K) / 127        # [1,N]   per column
    q_lhs = round(lhs / lhs_scale)  (int8 range)
    q_rhs = round(rhs / rhs_scale)
    out = (q_lhs @ q_rhs) * lhs_scale * rhs_scale

Sharding: data-parallel over B*S rows across 8 cores (4096 rows each);
rhs replicated (shipped pre-transposed [N,K] by the host). Per core:
  - lhs row-tile pairs [128,2,1024] f32 DMA in (1MB transfers);
    fused DVE absmax-reduce per pair
  - quantize: ACT (x*rs + M) per-partition scale, DVE (y - M)
    -> integer-valued bf16 (M = 1.5*2^23 magic gives round-half-even,
    matching jnp.round)
  - TensorE transpose (8 blocks into one bf16 PSUM bank) + one DVE
    copy -> lhsT [K,M] bf16 stationary tiles
  - TensorE bf16 matmuls accumulate over K in PSUM (exact: integer
    products up to 127*127 are exact in bf16*bf16->f32-accum)
  - per-column rhs scale folded into q_rhs (bf16): dequant is a single
    ScalarE pass out = psum * row_scale, then DMA out.
"""

import numpy as np

N_CORES = 8
B, S, K, N = 4, 8192, 1024, 1024
ROWS_TOTAL = B * S
ROWS_PER_CORE = ROWS_TOTAL // N_CORES  # 4096
P = 128
ROW_TILES = ROWS_PER_CORE // P  # 32
KT = K // P  # 8 k-tiles
NB = N // P  # 8 n-blocks
MAGIC = 12582912.0  # 1.5 * 2**23
INV127 = 1.0 / 127.0

_CACHE = {}


def _build():
    import concourse.mybir as mybir
    import concourse.tile as tile
    from concourse import bacc
    from concourse.masks import make_identity

    f32 = mybir.dt.float32
    bf16 = mybir.dt.bfloat16
    Copy = mybir.ActivationFunctionType.Copy

    nc = bacc.Bacc(None, target_bir_lowering=False, debug=False)
    lhs_d = nc.declare_dram_parameter("lhs", [ROWS_PER_CORE, K], f32, isOutput=False)
    # rhs is shipped PRE-TRANSPOSED from the host: [N, K] f32
    rhs_d = nc.declare_dram_parameter("rhs", [N, K], f32, isOutput=False)
    out_d = nc.declare_dram_parameter("out", [ROWS_PER_CORE, N], f32, isOutput=True)

    with tile.TileContext(nc) as tc:
        with (
            tc.tile_pool(name="qrhs", bufs=1) as qrhs_pool,
            tc.tile_pool(name="lhsx", bufs=6) as lhsx_pool,
            tc.tile_pool(name="scales", bufs=4) as scales_pool,
            tc.tile_pool(name="ybuf", bufs=6) as y_pool,
            tc.tile_pool(name="qbuf", bufs=6) as q_pool,
            tc.tile_pool(name="qlt", bufs=10) as qlt_pool,
            tc.tile_pool(name="obuf", bufs=3) as o_pool,
            tc.tile_pool(name="mm_psum", bufs=2, space="PSUM") as mm_psum,
            tc.tile_pool(name="tp_psum", bufs=2, space="PSUM") as tp_psum,
            tc.tile_pool(name="rhsbuf", bufs=2) as rhsbuf_pool,
            tc.tile_pool(name="rhs_misc", bufs=1) as rhs_misc,
        ):
            q_rhs = qrhs_pool.tile([P, KT, N], bf16, name="q_rhs")
            ident = qrhs_pool.tile([P, P], f32, name="ident")
            make_identity(nc, ident)
            identb = qrhs_pool.tile([P, P], bf16, name="identb")
            nc.vector.tensor_copy(out=identb, in_=ident)

            PAIRS = ROW_TILES // 2
            LA = 6
            staged = {}

            def stage_load(pi):
                xp = lhsx_pool.tile([P, 2, K], f32, name="xp", tag="xp")
                nc.scalar.dma_start(
                    out=xp,
                    in_=lhs_d[pi * 2 * P:(pi + 1) * 2 * P, :].rearrange(
                        "(t p) k -> p t k", p=P),
                )
                amax = scales_pool.tile([P, 2], f32, name="amax", tag="amax")
                nc.vector.tensor_reduce(
                    out=amax, in_=xp,
                    axis=mybir.AxisListType.X, op=mybir.AluOpType.max,
                    apply_absolute_value=True,
                )
                s_row = scales_pool.tile([P, 2], f32, name="s_row", tag="s_row")
                nc.vector.tensor_scalar_mul(s_row, amax, INV127)
                rs_row = scales_pool.tile([P, 2], f32, name="rs_row", tag="rs_row")
                nc.vector.reciprocal(rs_row, s_row)
                staged[pi] = (xp, s_row, rs_row)

            def stage_compute(pi):
                xp, s_row, rs_row = staged.pop(pi)
                o_pair = o_pool.tile([P, 2, N], f32, name="opair", tag="opair")
                for h in range(2):
                    y = y_pool.tile([P, K], f32, name="y", tag="y")
                    nc.scalar.activation(
                        out=y, in_=xp[:, h, :], func=Copy,
                        bias=MAGIC, scale=rs_row[:, h:h + 1],
                    )
                    q = q_pool.tile([P, K], bf16, name="q", tag="q")
                    nc.vector.tensor_scalar(
                        out=q, in0=y, scalar1=MAGIC, scalar2=None,
                        op0=mybir.AluOpType.subtract,
                    )
                    qlT = qlt_pool.tile([P, KT, P], bf16, name="qlT", tag="qlT")
                    psb = tp_psum.tile([P, KT * P], bf16, name="psb2", tag="psb2")
                    for b in range(KT):
                        nc.tensor.transpose(
                            psb[:, b * P:(b + 1) * P], q[:, b * P:(b + 1) * P],
                            identb)
                    nc.vector.tensor_copy(
                        out=qlT.rearrange("p b m -> p (b m)"), in_=psb)

                    ps0 = mm_psum.tile([P, 512], f32, name="ps0", tag="ps0")
                    ps1 = mm_psum.tile([P, 512], f32, name="ps1", tag="ps1")
                    for b in range(KT):
                        nc.tensor.matmul(
                            ps0, qlT[:, b, :], q_rhs[:, b, 0:512],
                            start=(b == 0), stop=(b == KT - 1),
                        )
                    for b in range(KT):
                        nc.tensor.matmul(
                            ps1, qlT[:, b, :], q_rhs[:, b, 512:1024],
                            start=(b == 0), stop=(b == KT - 1),
                        )
                    nc.scalar.activation(
                        out=o_pair[:, h, 0:512], in_=ps0, func=Copy,
                        bias=0.0, scale=s_row[:, h:h + 1],
                    )
                    nc.scalar.activation(
                        out=o_pair[:, h, 512:1024], in_=ps1, func=Copy,
                        bias=0.0, scale=s_row[:, h:h + 1],
                    )
                nc.sync.dma_start(
                    out=out_d[pi * 2 * P:(pi + 1) * 2 * P, :].rearrange(
                        "(t p) k -> p t k", p=P),
                    in_=o_pair,
                )

            # ------- rhs prep (replicated; interleaved with early lhs
            # loads so all engines ramp together) -------------------------
            s_col = rhs_misc.tile([P, NB], f32, name="s_col")
            rs_col = rhs_misc.tile([P, NB], f32, name="rs_col")
            namax = rhs_misc.tile([P, NB], f32, name="namax")
            rhsTs = []
            for nb in range(NB):
                rhsT_loc = rhsbuf_pool.tile(
                    [P, K], f32, name="rhsT_loc", tag="rhsT_loc", bufs=NB)
                nc.sync.dma_start(
                    out=rhsT_loc, in_=rhs_d[nb * P:(nb + 1) * P, :])
                nc.vector.tensor_reduce(
                    out=namax[:, nb:nb + 1], in_=rhsT_loc,
                    axis=mybir.AxisListType.X, op=mybir.AluOpType.max,
                    apply_absolute_value=True,
                )
                rhsTs.append(rhsT_loc)
            nc.vector.tensor_scalar_mul(s_col, namax, INV127)
            nc.vector.reciprocal(rs_col, s_col)

            def prep_quant(nb):
                yT = rhsbuf_pool.tile([P, K], f32, name="yp", tag="yp")
                nc.scalar.activation(
                    out=yT, in_=rhsTs[nb], func=Copy,
                    bias=MAGIC, scale=rs_col[:, nb:nb + 1],
                )
                qiT = rhsbuf_pool.tile([P, K], f32, name="yp", tag="yp")
                nc.vector.tensor_scalar(
                    out=qiT, in0=yT, scalar1=MAGIC, scalar2=None,
                    op0=mybir.AluOpType.subtract,
                )
                qsT = rhsbuf_pool.tile([P, K], bf16, name="qp", tag="qp")
                nc.scalar.activation(
                    out=qsT, in_=qiT, func=Copy,
                    bias=0.0, scale=s_col[:, nb:nb + 1],
                )
                psb = tp_psum.tile([P, KT * P], bf16, name="psb2", tag="psb2")
                for b in range(KT):
                    nc.tensor.transpose(
                        psb[:, b * P:(b + 1) * P], qsT[:, b * P:(b + 1) * P],
                        identb)
                nc.vector.tensor_copy(
                    out=q_rhs[:, :, nb * P:(nb + 1) * P],
                    in_=psb.rearrange("p (b m) -> p b m", b=KT))

            # interleave: early lhs loads between prep chains
            stage_load(0)
            stage_load(1)
            for nb in range(NB):
                prep_quant(nb)
                if nb % 2 == 1:
                    stage_load(2 + nb // 2)

            # ---------------- lhs main loop ------------------------------
            for ci in range(PAIRS):
                li = ci + LA
                if LA <= li < PAIRS:
                    stage_load(li)
                stage_compute(ci)

    nc.compile()
    return nc


def _get_nc():
    if "nc" not in _CACHE:
        _CACHE["nc"] = _build()
    return _CACHE["nc"]


def kernel(lhs: np.ndarray, rhs: np.ndarray) -> np.ndarray:
    from concourse.bass_utils import run_bass_kernel_spmd

    nc = _get_nc()
    lhs_flat = np.ascontiguousarray(lhs.reshape(ROWS_TOTAL, K), dtype=np.float32)
    rhsT = np.ascontiguousarray(np.asarray(rhs, dtype=np.float32).T)
    in_maps = [
        {
            "lhs": lhs_flat[c * ROWS_PER_CORE:(c + 1) * ROWS_PER_CORE],
            "rhs": rhsT,
        }
        for c in range(N_CORES)
    ]
    res = run_bass_kernel_spmd(nc, in_maps, core_ids=list(range(N_CORES)))
    out = np.concatenate([res.results[c]["out"] for c in range(N_CORES)], axis=0)
    return out.reshape(B, S, N)
